# revision 1
# baseline (speedup 1.0000x reference)
"""Trainium2 Bass kernel for the SNN leaky-integrate-and-fire problem.

Reference semantics (per batch row b, channels h=224, time t=224):
    x = roll(inp, 57, axis=time)
    T(b,t) = 3 + 2*tanh(dot(x[b,:,t], w))        (clip(1,5) is a no-op: 3+2*tanh is in [1,5])
    mem(t) = beta*mem(t-1) + x(t) - T(t)*[mem(t-1) > T(t)]
    spk(t) = [mem(t) > T(t)]
    out[b, 0, h, t] = spk

Sharding: pure data parallelism over batch (1024 -> 8 cores x 128). The
128-row batch shard maps exactly onto the 128 SBUF partitions; h rides the
free dimension, and the t recurrence runs as a sequence of [128, 224] vector
ops. w is replicated.
"""

import os
from contextlib import ExitStack

import numpy as np

import concourse.bass as bass
import concourse.tile as tile
from concourse import bacc, bass_utils, mybir

F32 = mybir.dt.float32
Alu = mybir.AluOpType
Act = mybir.ActivationFunctionType

CH = 224           # channels (h)
TT = 224           # time steps
ROLL = 57
BETA = 0.95
N_CORES = 8
BATCH = 1024
BPC = BATCH // N_CORES   # 128 = SBUF partitions


def _blocks(total, size):
    """[(t0, len)] covering range(total) in chunks of `size`."""
    out = []
    t0 = 0
    while t0 < total:
        out.append((t0, min(size, total - t0)))
        t0 += size
    return out


def _rolled_segments(t0, ln, total, roll):
    """DRAM time segments for rolled block [t0, t0+ln): x_rolled[t] = inp[(t - roll) % total].

    Returns [(dst_off, src_t0, seg_len)]."""
    src0 = (t0 - roll) % total
    if src0 + ln <= total:
        return [(0, src0, ln)]
    first = total - src0
    return [(0, src0, first), (first, 0, ln - first)]


def lif_kernel(ctx, tc, out, inp, w, b=BPC, ch=CH, tt=TT, roll=ROLL,
               tc_block=32, spk_engine="vector"):
    """Emit the LIF kernel body. out/inp/w are DRAM APs."""
    nc = tc.nc
    if True:
        pers = ctx.enter_context(tc.tile_pool(name="pers", bufs=1))
        xpool = ctx.enter_context(tc.tile_pool(name="x", bufs=2))
        spool = ctx.enter_context(tc.tile_pool(name="spk", bufs=2))
        tpool = ctx.enter_context(tc.tile_pool(name="thr", bufs=2))
        psum = ctx.enter_context(tc.tile_pool(name="ps", bufs=1, space="PSUM"))

        # ---- persistent state ----
        mem = pers.tile([b, ch], F32, tag="mem")    # membrane potential
        u = pers.tile([b, ch], F32, tag="u")        # beta*mem + x scratch
        rT = pers.tile([b, ch], F32, tag="rT")      # reset*T scratch
        junk = pers.tile([b, ch], F32, tag="junk")  # STT elementwise product sink
        wb = pers.tile([b, ch], F32, tag="wb")      # w broadcast to all partitions
        w_sb = pers.tile([1, ch], F32, tag="wsb")
        ones = pers.tile([1, b], F32, tag="ones")

        nc.vector.memset(mem[:], 0.0)
        nc.gpsimd.memset(ones[:], 1.0)
        nc.sync.dma_start(w_sb[:], w[None, :])

        # broadcast w to 128 partitions via a K=1 outer product on the PE
        wb_ps = psum.tile([b, ch], F32, tag="wbps")
        nc.tensor.matmul(wb_ps[:], ones[:], w_sb[:], start=True, stop=True)
        nc.scalar.copy(wb[:], wb_ps[:])

        spk_eng = getattr(nc, spk_engine)

        for t0, ln in _blocks(tt, tc_block):
            # ---- load x block (rolled time order), layout [b, ch, ln] ----
            xb = xpool.tile([b, ch, ln], F32, tag="x")
            for dst, src_t0, seg in _rolled_segments(t0, ln, tt, roll):
                nc.sync.dma_start(
                    xb[:, :, dst:dst + seg], inp[:, :, src_t0:src_t0 + seg]
                )

            # ---- thresholds for the block: T = 3 + 2*tanh(x_t . w) ----
            dots = tpool.tile([b, ln], F32, tag="dots")
            for tl in range(ln):
                nc.vector.scalar_tensor_tensor(
                    junk[:], xb[:, :, tl], 1.0, wb[:],
                    op0=Alu.mult, op1=Alu.mult,
                    accum_out=dots[:, tl:tl + 1],
                )
            tanh = tpool.tile([b, ln], F32, tag="tanh")
            nc.scalar.activation(tanh[:], dots[:], Act.Tanh)
            thr = tpool.tile([b, ln], F32, tag="thr")
            nc.vector.tensor_scalar(thr[:], tanh[:], 2.0, 3.0, op0=Alu.mult, op1=Alu.add)

            # ---- recurrence over the block ----
            spk = spool.tile([b, ch, ln], F32, tag="spk")
            for tl in range(ln):
                tcol = thr[:, tl:tl + 1]
                # rT = T * (mem > T)   (uses mem from step t-1)
                nc.vector.tensor_scalar(
                    rT[:], mem[:], tcol, tcol, op0=Alu.is_gt, op1=Alu.mult
                )
                # u = beta*mem + x_t   (matches reference association order)
                nc.vector.scalar_tensor_tensor(
                    u[:], mem[:], BETA, xb[:, :, tl], op0=Alu.mult, op1=Alu.add
                )
                # mem = u - rT
                nc.vector.tensor_sub(mem[:], u[:], rT[:])
                # spk_t = (mem > T)
                spk_eng.tensor_scalar(
                    spk[:, :, tl], mem[:], tcol, None, op0=Alu.is_gt
                )

            # ---- store spikes ----
            nc.sync.dma_start(out[:, 0, :, t0:t0 + ln], spk[:, :, :])


def build_kernel(b=BPC, ch=CH, tt=TT, roll=ROLL, tc_block=32, spk_engine="vector"):
    """Build the per-core Bass program. Returns the compiled Bacc object."""
    nc = bacc.Bacc()
    inp = nc.dram_tensor("inp", [b, ch, tt], F32, kind="ExternalInput")
    w = nc.dram_tensor("w", [ch], F32, kind="ExternalInput")
    out = nc.dram_tensor("out", [b, 1, ch, tt], F32, kind="ExternalOutput")

    with tile.TileContext(nc) as tc:
        with ExitStack() as ctx:
            lif_kernel(ctx, tc, out, inp, w, b=b, ch=ch, tt=tt, roll=roll,
                       tc_block=tc_block, spk_engine=spk_engine)

    nc.compile()
    return nc


_NC_CACHE = {}


def _get_nc():
    key = "default"
    if key not in _NC_CACHE:
        _NC_CACHE[key] = build_kernel()
    return _NC_CACHE[key]


def kernel(inp: np.ndarray, w: np.ndarray) -> np.ndarray:
    inp = np.ascontiguousarray(inp, dtype=np.float32)
    w = np.ascontiguousarray(w, dtype=np.float32)
    assert inp.shape == (BATCH, CH, TT) and w.shape == (CH,)

    nc = _get_nc()
    shards = np.split(inp, N_CORES, axis=0)
    in_maps = [{"inp": s, "w": w} for s in shards]
    trace = bool(int(os.environ.get("LIF_TRACE", "0")))
    res = bass_utils.run_bass_kernel_spmd(
        nc, in_maps, core_ids=list(range(N_CORES)), trace=trace
    )
    kernel.last_results = res
    outs = [r["out"] for r in res.results]
    return np.concatenate(outs, axis=0)



# revision 5
# speedup vs baseline: 1.1307x; 1.1307x over previous
"""Trainium2 Bass kernel for the SNN leaky-integrate-and-fire problem.

Reference semantics (per batch row b, channels h=224, time t=224):
    x = roll(inp, 57, axis=time)
    T(b,t) = 3 + 2*tanh(dot(x[b,:,t], w))        (clip(1,5) is a no-op)
    mem(t) = beta*mem(t-1) + x(t) - T(t)*[mem(t-1) > T(t)]
    spk(t) = [mem(t) > T(t)]
    out[b, 0, h, t] = spk

Sharding: pure data parallelism over batch (1024 -> 8 cores x 128); the
128-row shard maps onto the 128 SBUF partitions, h rides the free dim and
the t recurrence runs as a sequence of [128, 224] elementwise ops.

v2 layout: the host pre-rolls and re-blocks the input to
[b, NB, ch, TB] so every device DMA is one contiguous 28KB run per
partition (the naive [b, ch, t-slice] load costs 430k sub-512B DMA
packets and saturates the DMA engines). Spikes are produced as uint8 in
the same blocked layout and upcast to f32 on the host.

Engine split per time step: DVE runs the serial recurrence (reset, leak,
subtract); GPSIMD runs the threshold dot-products (for the *next* block,
interleaved) and the spike comparisons; the scalar engine runs tanh.
"""

import os
from contextlib import ExitStack

import numpy as np

import concourse.bass as bass
import concourse.tile as tile
from concourse import bacc, bass_utils, mybir

F32 = mybir.dt.float32
U8 = mybir.dt.uint8
Alu = mybir.AluOpType
Act = mybir.ActivationFunctionType

CH = 224           # channels (h)
TT = 224           # time steps
ROLL = 57
BETA = 0.95
N_CORES = 8
BATCH = 1024
BPC = BATCH // N_CORES   # 128 = SBUF partitions
TB = 32            # time block
NB = TT // TB


def lif_kernel(ctx, tc, out, inp, w, b=BPC, ch=CH, tb=TB, nb=NB,
               spk_engine="vector", dots_engine="vector", u_engine="vector"):
    """Emit the LIF kernel body.

    inp: [b, nb, ch, tb] f32 (host pre-rolled/blocked), w: [ch] f32,
    out: [b, nb, ch, tb] u8 spikes.
    """
    nc = tc.nc
    pers = ctx.enter_context(tc.tile_pool(name="pers", bufs=1))
    psum = ctx.enter_context(tc.tile_pool(name="ps", bufs=1, space="PSUM"))

    spk_eng = getattr(nc, spk_engine)
    dots_eng = getattr(nc, dots_engine)
    u_eng = getattr(nc, u_engine)

    # ---- persistent state ----
    mem = [pers.tile([b, ch], F32, tag=f"mem{i}", name=f"mem{i}") for i in range(2)]
    u = pers.tile([b, ch], F32, tag="u")
    rT = pers.tile([b, ch], F32, tag="rT")
    junk = pers.tile([b, ch], F32, tag="junk")   # dots STT elementwise sink
    wb = pers.tile([b, ch], F32, tag="wb")       # w broadcast to partitions
    w_sb = pers.tile([1, ch], F32, tag="wsb")
    ones = pers.tile([1, b], F32, tag="ones")
    xb = [pers.tile([b, ch, tb], F32, tag=f"xb{i}", name=f"xb{i}") for i in range(2)]
    spk = [pers.tile([b, ch, tb], U8, tag=f"spk{i}", name=f"spk{i}") for i in range(2)]
    dots = [pers.tile([b, tb], F32, tag=f"dots{i}", name=f"dots{i}") for i in range(2)]
    tanh = [pers.tile([b, tb], F32, tag=f"tanh{i}", name=f"tanh{i}") for i in range(2)]
    thr = [pers.tile([b, tb], F32, tag=f"thr{i}", name=f"thr{i}") for i in range(2)]

    nc.vector.memset(mem[1][:], 0.0)   # mem index: step t writes mem[t%2]
    nc.gpsimd.memset(ones[:], 1.0)
    nc.sync.dma_start(w_sb[:], w[None, :])

    # broadcast w to all partitions via a K=1 outer product on the PE
    wb_ps = psum.tile([b, ch], F32, tag="wbps")
    nc.tensor.matmul(wb_ps[:], ones[:], w_sb[:], start=True, stop=True)
    nc.scalar.copy(wb[:], wb_ps[:])

    def emit_dots(k, tl):
        """Threshold dot-products for block k, one time column."""
        dots_eng.scalar_tensor_tensor(
            junk[:], xb[k % 2][:, :, tl], 1.0, wb[:],
            op0=Alu.mult, op1=Alu.mult,
            accum_out=dots[k % 2][:, tl:tl + 1],
        )

    def emit_thr(k):
        """tanh + affine to turn dots into thresholds for block k."""
        nc.scalar.activation(tanh[k % 2][:], dots[k % 2][:], Act.Tanh)
        nc.vector.tensor_scalar(
            thr[k % 2][:], tanh[k % 2][:], 2.0, 3.0, op0=Alu.mult, op1=Alu.add
        )

    # prologue: load block 0, compute its thresholds
    nc.sync.dma_start(xb[0][:], inp[:, 0])
    for tl in range(tb):
        emit_dots(0, tl)
    emit_thr(0)

    t_glob = 0
    for k in range(nb):
        if k + 1 < nb:
            nc.sync.dma_start(xb[(k + 1) % 2][:], inp[:, k + 1])
        xcur = xb[k % 2]
        scur = spk[k % 2]
        tcur = thr[k % 2]
        for tl in range(tb):
            tcol = tcur[:, tl:tl + 1]
            mprev = mem[(t_glob + 1) % 2]
            mcur = mem[t_glob % 2]
            # rT = T * (mem > T)
            nc.vector.tensor_scalar(
                rT[:], mprev[:], tcol, tcol, op0=Alu.is_gt, op1=Alu.mult
            )
            # u = beta*mem + x_t
            u_eng.scalar_tensor_tensor(
                u[:], mprev[:], BETA, xcur[:, :, tl], op0=Alu.mult, op1=Alu.add
            )
            # mem' = u - rT
            nc.vector.tensor_sub(mcur[:], u[:], rT[:])
            # next block's dots ride in GPSIMD slack ahead of the spike
            if k + 1 < nb:
                emit_dots(k + 1, tl)
            # spk_t = (mem' > T)  (uint8 out)
            spk_eng.tensor_scalar(
                scur[:, :, tl], mcur[:], tcol, None, op0=Alu.is_gt
            )
            t_glob += 1
        if k + 1 < nb:
            emit_thr(k + 1)
        nc.sync.dma_start(out[:, k], scur[:])


def build_kernel(b=BPC, ch=CH, tb=TB, nb=NB, spk_engine="vector",
                 dots_engine="vector", u_engine="vector"):
    nc = bacc.Bacc()
    inp = nc.dram_tensor("inp", [b, nb, ch, tb], F32, kind="ExternalInput")
    w = nc.dram_tensor("w", [ch], F32, kind="ExternalInput")
    out = nc.dram_tensor("out", [b, nb, ch, tb], U8, kind="ExternalOutput")

    with tile.TileContext(nc) as tc:
        with ExitStack() as ctx:
            lif_kernel(ctx, tc, out, inp, w, b=b, ch=ch, tb=tb, nb=nb,
                       spk_engine=spk_engine, dots_engine=dots_engine,
                       u_engine=u_engine)

    nc.compile()
    return nc


def host_pack(inp):
    """[B, ch, t] f32 -> rolled, time-blocked [B, nb, ch, tb]."""
    xr = np.roll(inp, ROLL, axis=2)
    xb = xr.reshape(inp.shape[0], CH, NB, TB).transpose(0, 2, 1, 3)
    return np.ascontiguousarray(xb)


def host_unpack(out_u8):
    """[B, nb, ch, tb] u8 spikes -> [B, 1, ch, t] f32."""
    o = out_u8.transpose(0, 2, 1, 3).reshape(out_u8.shape[0], CH, TT)
    return o.astype(np.float32)[:, None]


_NC_CACHE = {}


def _get_nc():
    key = "default"
    if key not in _NC_CACHE:
        _NC_CACHE[key] = build_kernel()
    return _NC_CACHE[key]


def kernel(inp: np.ndarray, w: np.ndarray) -> np.ndarray:
    inp = np.ascontiguousarray(inp, dtype=np.float32)
    w = np.ascontiguousarray(w, dtype=np.float32)
    assert inp.shape == (BATCH, CH, TT) and w.shape == (CH,)

    nc = _get_nc()
    packed = host_pack(inp)
    shards = np.split(packed, N_CORES, axis=0)
    in_maps = [{"inp": s, "w": w} for s in shards]
    trace = bool(int(os.environ.get("LIF_TRACE", "0")))
    res = bass_utils.run_bass_kernel_spmd(
        nc, in_maps, core_ids=list(range(N_CORES)), trace=trace
    )
    kernel.last_results = res
    outs = [r["out"] for r in res.results]
    return host_unpack(np.concatenate(outs, axis=0))


# revision 7
# speedup vs baseline: 1.3453x; 1.1899x over previous
"""Trainium2 Bass kernel for the SNN leaky-integrate-and-fire problem.

Reference semantics (per batch row b, channels h=224, time t=224):
    x = roll(inp, 57, axis=time)
    T(b,t) = 3 + 2*tanh(dot(x[b,:,t], w))        (clip(1,5) is a no-op)
    mem(t) = beta*mem(t-1) + x(t) - T(t)*[mem(t-1) > T(t)]
    spk(t) = [mem(t) > T(t)]
    out[b, 0, h, t] = spk

Sharding: pure data parallelism over batch (1024 -> 8 cores x 128); the
128-row shard maps onto the 128 SBUF partitions, h rides the free dim and
the t recurrence runs as a sequence of [128, 224] elementwise ops.

Layout: the host pre-rolls and re-blocks the input to [b, NB, ch, TB] so
every device DMA is one contiguous run per partition (a naive
[b, ch, t-slice] load costs 430k sub-512B DMA packets and saturates the
DMA engines). Spikes leave the device as uint8 in the same blocked layout
and are upcast to f32 on the host.

Engine split: DVE runs only the serial 3-op recurrence (reset, leak,
subtract). The threshold dot-products run on the tensor engine as tiny
[ch,b]x[ch,1] matmuls from a host-shipped channel-major copy of x,
accumulated per block in PSUM. The scalar engine reads the PSUM dots
straight into tanh, and computes each spike column as uint8 via
Sign(mem - T) (the float->uint8 store saturates -1 to 0).
"""

import os
from contextlib import ExitStack

import numpy as np

import concourse.bass as bass
import concourse.tile as tile
from concourse import bacc, bass_utils, mybir

F32 = mybir.dt.float32
U8 = mybir.dt.uint8
Alu = mybir.AluOpType
Act = mybir.ActivationFunctionType

CH = 224           # channels (h)
TT = 224           # time steps
ROLL = 57
BETA = 0.95
N_CORES = 8
BATCH = 1024
BPC = BATCH // N_CORES   # 128 = SBUF partitions
TB = 32            # time block
NB = TT // TB
KC = CH // 2       # PE contraction chunk (112 <= 128 partitions)
BIG = float(2.0 ** 100)  # exact power-of-two spike sharpener


def lif_kernel(ctx, tc, out, inp, inpT, w, b=BPC, ch=CH, tb=TB, nb=NB,
               spk_mode="sign", dots_mode="pe"):
    """Emit the LIF kernel body.

    inp:  [b, nb, ch, tb] f32  (host pre-rolled/blocked, batch-major)
    inpT: [nb, 2, KC, tb, b] f32 (same data, channel-major for the PE)
    w:    [ch] f32
    out:  [b, nb, ch, tb] u8 spikes
    """
    nc = tc.nc
    pers = ctx.enter_context(tc.tile_pool(name="pers", bufs=1))
    psum = ctx.enter_context(tc.tile_pool(name="ps", bufs=1, space="PSUM"))

    # ---- persistent state ----
    mem = [pers.tile([b, ch], F32, tag=f"mem{i}", name=f"mem{i}")
           for i in range(2)]
    u = pers.tile([b, ch], F32, tag="u")
    rT = pers.tile([b, ch], F32, tag="rT")
    xb = [pers.tile([b, ch, tb], F32, tag=f"xb{i}", name=f"xb{i}")
          for i in range(2)]
    xT = [pers.tile([KC, 2, tb, b], F32, tag=f"xT{i}", name=f"xT{i}")
          for i in range(2)]
    spk = [pers.tile([b, ch, tb], U8, tag=f"spk{i}", name=f"spk{i}")
           for i in range(2)]
    tanh = [pers.tile([b, tb], F32, tag=f"tanh{i}", name=f"tanh{i}")
            for i in range(2)]
    thr = [pers.tile([b, tb], F32, tag=f"thr{i}", name=f"thr{i}")
           for i in range(2)]
    nthr = [pers.tile([b, tb], F32, tag=f"nthr{i}", name=f"nthr{i}")
            for i in range(2)]
    wc = pers.tile([KC, 2], F32, tag="wc")       # w chunks, one per column
    dots_ps = [psum.tile([b, tb], F32, tag=f"dps{i}", name=f"dps{i}")
               for i in range(2)]

    # dots fallback (STT on DVE) support
    junk = pers.tile([b, ch], F32, tag="junk")
    wb = pers.tile([b, ch], F32, tag="wb")
    w_sb = pers.tile([1, ch], F32, tag="wsb")
    ones = pers.tile([1, b], F32, tag="ones")
    dots_sb = [pers.tile([b, tb], F32, tag=f"dsb{i}", name=f"dsb{i}")
               for i in range(2)]

    nc.vector.memset(mem[1][:], 0.0)   # step t writes mem[t%2]
    nc.sync.dma_start(wc[:, 0:1], w[0:KC][:, None])
    nc.sync.dma_start(wc[:, 1:2], w[KC:ch][:, None])
    if dots_mode == "stt":
        nc.gpsimd.memset(ones[:], 1.0)
        nc.sync.dma_start(w_sb[:], w[None, :])
        wb_ps = psum.tile([b, ch], F32, tag="wbps")
        nc.tensor.matmul(wb_ps[:], ones[:], w_sb[:], start=True, stop=True)
        nc.scalar.copy(wb[:], wb_ps[:])

    def emit_dots(k, tl):
        """Threshold dot-products for block k, one time column."""
        if dots_mode == "pe":
            for c in range(2):
                nc.tensor.matmul(
                    dots_ps[k % 2][:, tl:tl + 1],
                    xT[k % 2][:, c, tl, :],
                    wc[:, c:c + 1],
                    start=(c == 0), stop=(c == 1),
                )
        else:
            nc.vector.scalar_tensor_tensor(
                junk[:], xb[k % 2][:, :, tl], 1.0, wb[:],
                op0=Alu.mult, op1=Alu.mult,
                accum_out=dots_sb[k % 2][:, tl:tl + 1],
            )

    def emit_thr(k):
        """tanh + affine to turn dots into thresholds for block k."""
        src = dots_ps[k % 2] if dots_mode == "pe" else dots_sb[k % 2]
        nc.scalar.activation(tanh[k % 2][:], src[:], Act.Tanh)
        nc.vector.tensor_scalar(
            thr[k % 2][:], tanh[k % 2][:], 2.0, 3.0, op0=Alu.mult, op1=Alu.add
        )
        if spk_mode == "sign":
            # exact 2^100 scaling of the rounded thr (see spike op below)
            nc.vector.tensor_scalar(
                nthr[k % 2][:], thr[k % 2][:], -BIG, None, op0=Alu.mult
            )

    def load_block(k):
        nc.sync.dma_start(xb[k % 2][:], inp[:, k])
        if dots_mode == "pe":
            nc.sync.dma_start(xT[k % 2][:, 0], inpT[k, 0])
            nc.sync.dma_start(xT[k % 2][:, 1], inpT[k, 1])

    # prologue: load block 0, compute its thresholds
    load_block(0)
    for tl in range(tb):
        emit_dots(0, tl)
    emit_thr(0)

    t_glob = 0
    for k in range(nb):
        if k + 1 < nb:
            load_block(k + 1)
        xcur = xb[k % 2]
        scur = spk[k % 2]
        tcur = thr[k % 2]
        ncur = nthr[k % 2]
        for tl in range(tb):
            tcol = tcur[:, tl:tl + 1]
            mprev = mem[(t_glob + 1) % 2]
            mcur = mem[t_glob % 2]
            # rT = T * (mem > T)
            nc.vector.tensor_scalar(
                rT[:], mprev[:], tcol, tcol, op0=Alu.is_gt, op1=Alu.mult
            )
            # u = beta*mem + x_t
            nc.vector.scalar_tensor_tensor(
                u[:], mprev[:], BETA, xcur[:, :, tl], op0=Alu.mult, op1=Alu.add
            )
            # mem' = u - rT
            nc.vector.tensor_sub(mcur[:], u[:], rT[:])
            # next block's dots ride on the PE in parallel
            if k + 1 < nb:
                emit_dots(k + 1, tl)
            # spk_t = (mem' > T) as uint8
            if spk_mode == "sign":
                # Sigmoid(2^100*(mem - thr)): both products are exact
                # (power-of-two scale), so the sign matches mem > thr
                # bit-for-bit; any nonzero f32 difference saturates the
                # sigmoid to exactly 0.0/1.0, which the u8 store keeps.
                nc.scalar.activation(
                    scur[:, :, tl], mcur[:], Act.Sigmoid,
                    bias=ncur[:, tl:tl + 1], scale=BIG,
                )
            else:
                nc.vector.tensor_scalar(
                    scur[:, :, tl], mcur[:], tcol, None, op0=Alu.is_gt
                )
            t_glob += 1
        if k + 1 < nb:
            emit_thr(k + 1)
        nc.sync.dma_start(out[:, k], scur[:])


def build_kernel(b=BPC, ch=CH, tb=TB, nb=NB, spk_mode="sign", dots_mode="pe"):
    nc = bacc.Bacc()
    inp = nc.dram_tensor("inp", [b, nb, ch, tb], F32, kind="ExternalInput")
    inpT = nc.dram_tensor("inpT", [nb, 2, KC, tb, b], F32,
                          kind="ExternalInput")
    w = nc.dram_tensor("w", [ch], F32, kind="ExternalInput")
    out = nc.dram_tensor("out", [b, nb, ch, tb], U8, kind="ExternalOutput")

    with tile.TileContext(nc) as tc:
        with ExitStack() as ctx:
            lif_kernel(ctx, tc, out, inp, inpT, w, b=b, ch=ch, tb=tb, nb=nb,
                       spk_mode=spk_mode, dots_mode=dots_mode)

    nc.compile()
    return nc


def host_pack(inp):
    """[B, ch, t] f32 -> rolled, time-blocked [B, nb, ch, tb]."""
    xr = np.roll(inp, ROLL, axis=2)
    xb = xr.reshape(inp.shape[0], CH, NB, TB).transpose(0, 2, 1, 3)
    return np.ascontiguousarray(xb)


def host_pack_T(packed):
    """[B, nb, ch, tb] (one shard) -> channel-major [nb, 2, KC, tb, B]."""
    xt = packed.transpose(1, 2, 3, 0)            # [nb, ch, tb, B]
    xt = xt.reshape(NB, 2, KC, TB, packed.shape[0])
    return np.ascontiguousarray(xt)


def host_unpack(out_u8):
    """[B, nb, ch, tb] u8 spikes -> [B, 1, ch, t] f32."""
    o = out_u8.transpose(0, 2, 1, 3).reshape(out_u8.shape[0], CH, TT)
    return o.astype(np.float32)[:, None]


_NC_CACHE = {}


def _get_nc():
    key = "default"
    if key not in _NC_CACHE:
        _NC_CACHE[key] = build_kernel()
    return _NC_CACHE[key]


def kernel(inp: np.ndarray, w: np.ndarray) -> np.ndarray:
    inp = np.ascontiguousarray(inp, dtype=np.float32)
    w = np.ascontiguousarray(w, dtype=np.float32)
    assert inp.shape == (BATCH, CH, TT) and w.shape == (CH,)

    nc = _get_nc()
    packed = host_pack(inp)
    shards = np.split(packed, N_CORES, axis=0)
    in_maps = [{"inp": s, "inpT": host_pack_T(s), "w": w} for s in shards]
    trace = bool(int(os.environ.get("LIF_TRACE", "0")))
    res = bass_utils.run_bass_kernel_spmd(
        nc, in_maps, core_ids=list(range(N_CORES)), trace=trace
    )
    kernel.last_results = res
    outs = [r["out"] for r in res.results]
    return host_unpack(np.concatenate(outs, axis=0))
